# revision 56
# baseline (speedup 1.0000x reference)
"""Trainium2 Bass kernel for nn_Decoder_76974403879078.

2-layer LSTM decoder, B=256, H=512, T=64 steps, argmax feedback.
Sharding: data-parallel over batch, 8 cores x 32.

Device design (per core, batch M=32):
  - All matmul operands bf16 (1 cycle/row on the PE vs 4 for fp32);
    accumulation stays fp32 in PSUM.  Elementwise/LSTM state fp32.
    Host-side bf16 study: rel err 3.7e-4 vs fp32 reference (tolerance 2e-2).
  - Moving operand = weights, stationary = h^T k-tiles [128,32]; 4-way PE
    column tiling writes the four gate chunks to PSUM partition groups
    32j..32j+32.
  - Gate layout "interleaved": chunk j (partitions 32j:32j+32) holds free
    cols [i|f|o|g] for hidden slice 128j:128(j+1), 128 each.
  - Each gate PSUM is a PAIR of tiles: Ga = i,f cols (0:256), Gb = o,g
    (256:512).  Dependency tracking is PSUM-tile-granular, so splitting
    lets the i/f sigmoid fire as soon as the a-half closes (~400ns before
    the full gate matmul finishes); the closing round orders a-halves
    first.  All gate rounds issue as [128,256] half-MMs (same streamed
    rows, so same PE time).
  - x-path folded: x_t = [emb[a], dur] enters layer 1 via a K=34 matmul with
    stationary onehot^T (+dur row +bias row) against a host-precomputed
    E1ext = [emb@Wih1[:,:511]^T ; Wih1[:,511] ; bih1+bhh1] table.
  - Constant injects (layer-2 gate bias B2G, conditionals path CC2G, fc3
    bias F3rep) are DVE copies into "warm" PSUM banks: after a bank's first
    accumulation group its has_written bits stay set, so start=False
    matmuls accumulate onto engine-written seeds (verified on HW).  The
    first two steps (cold banks) use identity-matmul injects instead.
    Saves ~1.1us/step of PE inject rounds.
  - Program order tuned for the greedy ready-first scheduler: fc2 is
    issued before the next-step G1 h-rounds so the argmax critical path
    preempts fill work; the Whh2-h2 rounds are the fill reserved for the
    L1-chain window.
  - argmax feedback: DVE max -> tensor_scalar is_equal -> DVE 32x32 transpose
    gives onehot^T for the next step's K=34 matmul.
  - log_softmax / time-softmax postprocess: chunked exp/reduce gated on the
    last step (a zero bias AP blocks scheduler hoisting, which would thrash
    ACT function tables); the exp-table switch is triggered right after the
    loop's last sigmoid so the 1.3us table load hides under PE work;
    output DMAd in contiguous 16-step chunks overlapping the subtracts.
  - weights stored in DRAM as bf16 (6.3MB/core); per-k-tile weight tiles
    DMAd in first-use order across three engine queues so the time loop
    starts after ~1MB instead of the full load.

Measured (CoreSim TRN2 cost model, per core): 820.4us vs 3439.6us for the
fp32 predecessor (4.2x).  PE busy 95%.  Backend-validated rel err 3.74e-4.
"""
import sys
import numpy as np

sys.path.insert(0, "/opt/trn_rl_repo")

import os
HIDDEN = 512
OUT = 33
T_STEPS = int(os.environ.get("KERNEL_STEPS", "64"))
B_FULL = 256
N_CORES = 8
B = B_FULL // N_CORES  # 32
SLOPE = 0.01

_PROGRAM_CACHE = {}
LAST_EXEC_NS = None

# tensors stored/streamed as bf16 on device
_BF16_NAMES = {"Whh1p", "Wih2p", "Whh2p", "E1extp", "B2rep", "fc2Wp",
               "fc3Wp", "F3rep", "I128", "oh0T", "h1T0", "h2T0", "CC2p",
               "B2G", "CC2G"}


def _bf16np():
    import ml_dtypes
    return ml_dtypes.bfloat16


def _gate_perm():
    """perm[j*512 + q*128 + n] = original gate-row index.

    chunk j free layout: [i(0:128) | f(128:256) | o(256:384) | g(384:512)],
    hidden slice j = 128j..128(j+1).  torch gate order in W: i,f,g,o.
    """
    base = {"i": 0, "f": 512, "g": 1024, "o": 1536}
    perm = []
    for j in range(4):
        for q in ("i", "f", "o", "g"):
            perm.extend(range(base[q] + 128 * j, base[q] + 128 * (j + 1)))
    return np.asarray(perm, np.int64)


def _pack_w(Wl, perm):
    """[2048,512] gate weight -> rhs tile [128, 4(k-tile), 2048(perm cols)]."""
    wt = Wl[perm].T.astype(np.float32)            # [512 hidden, 2048 perm]
    return np.ascontiguousarray(
        wt.reshape(4, 128, 2048).transpose(1, 0, 2))  # [128,4,2048]


def _prep(inputs):
    """Host-side packing. Returns dict of global consts + per-core arrays."""
    f32 = np.float32
    emb = np.asarray(inputs["emb"], f32)
    Wih = np.asarray(inputs["Wih"], f32)
    Whh = np.asarray(inputs["Whh"], f32)
    bih = np.asarray(inputs["bih"], f32)
    bhh = np.asarray(inputs["bhh"], f32)
    fcW = np.asarray(inputs["fcW"], f32)
    fcb = np.asarray(inputs["fcb"], f32)
    fc2W = np.asarray(inputs["fc2W"], f32)
    fc2b = np.asarray(inputs["fc2b"], f32)
    fc3W = np.asarray(inputs["fc3W"], f32)
    fc3b = np.asarray(inputs["fc3b"], f32)
    h0 = np.asarray(inputs["h0"], f32)
    c0 = np.asarray(inputs["c0"], f32)
    conditionals = np.asarray(inputs["conditionals"], f32)

    perm = _gate_perm()
    g = {}
    g["Whh1p"] = _pack_w(Whh[0], perm)
    g["Wih2p"] = _pack_w(Wih[1], perm)
    g["Whh2p"] = _pack_w(Whh[1], perm)

    Wih1perm = Wih[0][perm]                       # [2048, 512]
    e1 = np.zeros((34, 2048), f32)
    e1[:32] = emb @ Wih1perm[:, :511].T
    e1[32] = Wih1perm[:, 511]
    e1[33] = (bih[0] + bhh[0])[perm]
    g["E1extp"] = e1

    b2p = (bih[1] + bhh[1])[perm].astype(f32)
    g["B2rep"] = np.tile(b2p[None, :], (B, 1)).astype(f32)
    # bias pre-laid-out in G2 PSUM shape: row 32j+b = bias[512j:512j+512]
    g["B2G"] = np.ascontiguousarray(
        np.repeat(b2p.reshape(4, 1, 512), B, axis=1).reshape(128, 512))

    g["fc2Wp"] = np.ascontiguousarray(
        fc2W.T.reshape(4, 128, 4, 128).transpose(1, 0, 2, 3))   # [128,4,4,128]
    # leaky(x) computed on device as 0.505x + 0.495|x| = 0.505*(x + k|x|);
    # the 0.505 is folded here into fc3W
    g["fc3Wp"] = np.ascontiguousarray(
        fc3W.T.reshape(4, 128, OUT).transpose(1, 0, 2)) * np.float32(0.505)
    g["F3rep"] = np.tile(fc3b[None, :], (B, 1)).astype(f32)

    g["I128"] = np.eye(128, dtype=f32)

    oh0 = np.zeros((34, B), f32)
    oh0[0, :] = 1.0   # SOS = 0
    oh0[32, :] = 0.0  # dur at t=0
    oh0[33, :] = 1.0  # bias row
    g["oh0T"] = oh0

    cond = conditionals @ fcW.T + fcb
    cond = np.where(cond >= 0, cond, SLOPE * cond).astype(f32)
    CC2 = (cond @ fc2W.T + fc2b).astype(f32)      # [256, 512]

    per_core = []
    for ci in range(N_CORES):
        sl = slice(ci * B, (ci + 1) * B)
        pc = {}
        for l, name in ((0, "h1T0"), (1, "h2T0")):
            hc = h0[l, sl]                         # [32, 512]
            pc[name] = np.ascontiguousarray(
                hc.reshape(B, 4, 128).transpose(2, 1, 0).reshape(128, 128))
        for l, name in ((0, "c10"), (1, "c20")):
            cc = c0[l, sl]
            pc[name] = np.ascontiguousarray(
                cc.reshape(B, 4, 128).transpose(1, 0, 2).reshape(128, 128))
        pc["CC2p"] = np.ascontiguousarray(CC2[sl].reshape(B, 4, 128))
        # CC2 pre-laid-out in fc2 PSUM shape [32j+b, m]
        pc["CC2G"] = np.ascontiguousarray(
            CC2[sl].reshape(B, 4, 128).transpose(1, 0, 2).reshape(128, 128))
        per_core.append(pc)
    return g, per_core


# ---------------------------------------------------------------------------
# numpy emulation of the exact device algorithm (for layout validation)
# ---------------------------------------------------------------------------
def _emulate_core(g, pc):
    f32 = np.float32

    def sig(x):
        return (1.0 / (1.0 + np.exp(-x))).astype(f32)

    h1T, h2T = pc["h1T0"].copy(), pc["h2T0"].copy()
    c1, c2 = pc["c10"].copy(), pc["c20"].copy()
    ohT = g["oh0T"].copy()
    preds = np.zeros((B, T_STEPS, OUT), f32)

    for t in range(T_STEPS):
        for layer in range(2):
            G = np.zeros((128, 512), f32)
            if layer == 0:
                Wp = g["Whh1p"]
                hT = h1T
                for j in range(4):
                    G[32 * j:32 * (j + 1)] += ohT.T @ g["E1extp"][:, 512 * j:512 * (j + 1)]
            else:
                Wp = g["Whh2p"]
                hT = h2T
                for j in range(4):
                    G[32 * j:32 * (j + 1)] += g["B2rep"][:, 512 * j:512 * (j + 1)]
                    for r in range(4):
                        G[32 * j:32 * (j + 1)] += (
                            h1T[:, 32 * r:32 * (r + 1)].T @ g["Wih2p"][:, r, 512 * j:512 * (j + 1)])
            for j in range(4):
                for r in range(4):
                    G[32 * j:32 * (j + 1)] += (
                        hT[:, 32 * r:32 * (r + 1)].T @ Wp[:, r, 512 * j:512 * (j + 1)])
            sg = np.empty_like(G)
            sg[:, 0:384] = sig(G[:, 0:384])
            sg[:, 384:512] = np.tanh(G[:, 384:512])
            c = c1 if layer == 0 else c2
            cn = sg[:, 128:256] * c + sg[:, 0:128] * sg[:, 384:512]
            hp = sg[:, 256:384] * np.tanh(cn)
            hT_new = np.zeros((128, 128), f32)
            for j in range(4):
                hT_new[:, 32 * j:32 * (j + 1)] = hp[32 * j:32 * (j + 1), :].T
            if layer == 0:
                c1, h1T = cn, hT_new
            else:
                c2, h2T = cn, hT_new
        # fc2 (packed out [128,128]) + CC2
        f = np.zeros((128, 128), f32)
        for j in range(4):
            f[32 * j:32 * (j + 1)] = pc["CC2p"][:, j, :]
            for r in range(4):
                f[32 * j:32 * (j + 1)] += (
                    h2T[:, 32 * r:32 * (r + 1)].T @ g["fc2Wp"][:, r, j, :])
        K_ABS = f32(0.495 / 0.505)
        y = (f + K_ABS * np.abs(f)).astype(f32)
        yT = np.zeros((128, 128), f32)
        for j in range(4):
            yT[:, 32 * j:32 * (j + 1)] = y[32 * j:32 * (j + 1), :].T
        pred = g["F3rep"].copy()
        for r in range(4):
            pred += yT[:, 32 * r:32 * (r + 1)].T @ g["fc3Wp"][:, r, :]
        preds[:, t, :] = pred
        if t < T_STEPS - 1:
            mx = pred[:, :32].max(1, keepdims=True)
            oh = (pred[:, :32] == mx).astype(f32)
            ohT[0:32, :] = oh.T
            ohT[32, :] = 1.0
            ohT[33, :] = 1.0
    # postprocess
    e = np.exp(preds)
    s = e[:, :, :32].sum(-1)
    logp = preds[:, :, :32] - np.log(s)[:, :, None]
    sd = e[:, :, 32].sum(1, keepdims=True)
    dur = e[:, :, 32] / sd
    return np.concatenate([logp, dur[:, :, None]], axis=-1).astype(np.float32)


def emulate(inputs):
    g, per_core = _prep(inputs)
    outs = [_emulate_core(g, pc) for pc in per_core]
    return np.concatenate(outs, axis=0)


# ---------------------------------------------------------------------------
# Bass program
# ---------------------------------------------------------------------------
def _build_program():
    import concourse.bass as bass
    import concourse.tile as tile
    from concourse import mybir, bacc

    F32 = mybir.dt.float32
    BF16 = mybir.dt.bfloat16
    AF = mybir.ActivationFunctionType
    ALU = mybir.AluOpType

    nc = bacc.Bacc("TRN2", target_bir_lowering=False, debug=False)

    def din(name, shape):
        dt = BF16 if name in _BF16_NAMES else F32
        return nc.dram_tensor(name, list(shape), dt, kind="ExternalInput").ap()

    d = {
        "Whh1p": din("Whh1p", (128, 4, 2048)),
        "Wih2p": din("Wih2p", (128, 4, 2048)),
        "Whh2p": din("Whh2p", (128, 4, 2048)),
        "E1extp": din("E1extp", (34, 2048)),
        "B2rep": din("B2rep", (B, 2048)),
        "fc2Wp": din("fc2Wp", (128, 4, 4, 128)),
        "fc3Wp": din("fc3Wp", (128, 4, OUT)),
        "F3rep": din("F3rep", (B, OUT)),
        "I128": din("I128", (128, 128)),
        "oh0T": din("oh0T", (34, B)),
        "h1T0": din("h1T0", (128, 128)),
        "h2T0": din("h2T0", (128, 128)),
        "c10": din("c10", (128, 128)),
        "c20": din("c20", (128, 128)),
        "CC2p": din("CC2p", (B, 4, 128)),
        "B2G": din("B2G", (128, 512)),
        "CC2G": din("CC2G", (128, 128)),
    }
    out_d = nc.dram_tensor("out", [B, 64, OUT], F32, kind="ExternalOutput").ap()

    with tile.TileContext(nc) as tc:
        import contextlib
        ctx = contextlib.ExitStack()
        with ctx:
            consts = ctx.enter_context(tc.tile_pool(name="consts", bufs=1))
            state = ctx.enter_context(tc.tile_pool(name="state", bufs=1))
            work = ctx.enter_context(tc.tile_pool(name="work", bufs=2))
            hpool = ctx.enter_context(tc.tile_pool(name="hpool", bufs=2))
            ps_g1a = ctx.enter_context(tc.tile_pool(name="ps_g1a", bufs=1, space="PSUM"))
            ps_g1b = ctx.enter_context(tc.tile_pool(name="ps_g1b", bufs=1, space="PSUM"))
            ps_g2a = ctx.enter_context(tc.tile_pool(name="ps_g2a", bufs=1, space="PSUM"))
            ps_g2b = ctx.enter_context(tc.tile_pool(name="ps_g2b", bufs=1, space="PSUM"))
            ps_fc2 = ctx.enter_context(tc.tile_pool(name="ps_fc2", bufs=2, space="PSUM"))
            ps_sm = ctx.enter_context(tc.tile_pool(name="ps_sm", bufs=1, space="PSUM"))
            ps_ht = ctx.enter_context(tc.tile_pool(name="ps_ht", bufs=1, space="PSUM"))

            # ---- constant tiles (bf16 streams, fp32 state) ----
            # gate weights split per k-tile so each DMA chunk unblocks its
            # own rounds (tile-granular dependencies)
            Whh1p = [consts.tile([128, 2048], BF16, name=f"Whh1p{r}")
                     for r in range(4)]
            Wih2p = [consts.tile([128, 2048], BF16, name=f"Wih2p{r}")
                     for r in range(4)]
            Whh2p = [consts.tile([128, 2048], BF16, name=f"Whh2p{r}")
                     for r in range(4)]
            E1extp = consts.tile([34, 2048], BF16)
            B2rep = consts.tile([B, 2048], BF16)
            B2G = consts.tile([128, 512], BF16)
            CC2G = consts.tile([128, 128], BF16)
            fc2Wp = consts.tile([128, 4, 4, 128], BF16)
            fc3Wp = consts.tile([128, 4, OUT], BF16)
            F3rep = consts.tile([B, OUT], BF16)
            I128 = consts.tile([128, 128], BF16)
            oh0T = consts.tile([34, B], BF16)
            CC2p = consts.tile([B, 4, 128], BF16)

            c1 = state.tile([128, 128], F32, tag="c1")
            c2 = state.tile([128, 128], F32, tag="c2")
            h1T = hpool.tile([128, 128], BF16, tag="h1T")
            h2T = hpool.tile([128, 128], BF16, tag="h2T")

            # DMAs ordered by first use AND spread across three engine
            # queues so the three weight streams load concurrently.
            nc.sync.dma_start(Whh1p[0][:], d["Whh1p"][:, 0])
            nc.sync.dma_start(h1T[:], d["h1T0"])
            nc.sync.dma_start(oh0T[:], d["oh0T"])
            nc.sync.dma_start(E1extp[:], d["E1extp"])
            nc.sync.dma_start(c1[:], d["c10"])
            for r in range(1, 4):
                nc.sync.dma_start(Whh1p[r][:], d["Whh1p"][:, r])
            nc.gpsimd.dma_start(B2rep[:], d["B2rep"])
            nc.gpsimd.dma_start(h2T[:], d["h2T0"])
            nc.gpsimd.dma_start(c2[:], d["c20"])
            for r in range(4):
                nc.gpsimd.dma_start(Whh2p[r][:], d["Whh2p"][:, r])
            nc.scalar.dma_start(I128[:], d["I128"])
            nc.scalar.dma_start(CC2p[:], d["CC2p"])
            nc.scalar.dma_start(F3rep[:], d["F3rep"])
            nc.scalar.dma_start(B2G[:], d["B2G"])
            nc.scalar.dma_start(CC2G[:], d["CC2G"])
            for r in range(4):
                nc.scalar.dma_start(Wih2p[r][:], d["Wih2p"][:, r])
            nc.scalar.dma_start(fc2Wp[:], d["fc2Wp"])
            nc.scalar.dma_start(fc3Wp[:], d["fc3Wp"])

            ohT = state.tile([34, B], BF16, tag="ohT")
            nc.vector.memset(ohT[32:34, :], 1.0)

            predbuf = state.tile([B, 64, OUT], F32, tag="predbuf")
            if T_STEPS < 64:
                nc.vector.memset(predbuf[:], 0.0)

            def col_round(psum, lhsT, rhs_fn, start, stop):
                for j in range(4):
                    nc.tensor.matmul(
                        psum[32 * j:32 * (j + 1), :], lhsT, rhs_fn(j),
                        start=start, stop=stop, tile_position=(0, 32 * j),
                        skip_group_check=True,
                    )

            def col_round2(Ga, Gb, lhsT, rhs_fn, start, stop, close=False):
                """One k-round split into a (gate cols 0:256 = i,f) and b
                (256:512 = o,g) half-MMs targeting separate PSUM tiles, so
                the sigmoid's dependency cone closes with the a-half.
                close=True orders all a-halves first."""
                order = ([(h, j) for h in (0, 1) for j in range(4)] if close
                         else [(h, j) for j in range(4) for h in (0, 1)])
                for h, j in order:
                    nc.tensor.matmul(
                        (Ga if h == 0 else Gb)[32 * j:32 * (j + 1), :], lhsT,
                        rhs_fn(j, h), start=start, stop=stop,
                        tile_position=(0, 32 * j), skip_group_check=True)

            def wslice(W):
                return lambda j, h: W[:, 512 * j + 256 * h:512 * j + 256 * (h + 1)]

            def nonlin(layer, Ga, Gb, c_own):
                """sigmoid/tanh + c/h update + transpose; returns new h^T.

                The PSUM->SBUF evacuation of h^T is split into column
                slices so the first dependent matmul round starts early.
                """
                sg = work.tile([128, 512], F32, tag=f"sg{layer}")
                # i,f sigmoid first, then tanh(g) (unblocks t1/t2 sooner);
                # the o-gate sigmoid is only needed later for hp
                nc.scalar.activation(sg[:, 0:256], Ga[:], AF.Sigmoid)
                nc.scalar.activation(sg[:, 384:512], Gb[:, 128:256], AF.Tanh)
                nc.scalar.activation(sg[:, 256:384], Gb[:, 0:128], AF.Sigmoid)
                t1 = work.tile([128, 128], F32, tag="t1")
                t2 = work.tile([128, 128], F32, tag="t2")
                nc.vector.tensor_tensor(t1[:], sg[:, 0:128], sg[:, 384:512], ALU.mult)
                nc.vector.tensor_tensor(t2[:], sg[:, 128:256], c_own[:], ALU.mult)
                nc.vector.tensor_tensor(c_own[:], t1[:], t2[:], ALU.add)
                tc_t = work.tile([128, 128], F32, tag="tc")
                nc.scalar.activation(tc_t[:], c_own[:], AF.Tanh)
                hp = work.tile([128, 128], BF16, tag=f"hp{layer}")
                nc.vector.tensor_tensor(hp[:], sg[:, 256:384], tc_t[:], ALU.mult)
                htp = ps_ht.tile([128, 128], BF16, tag="htp")
                nc.tensor.matmul(htp[:], hp[:], I128[:], is_transpose=True,
                                 skip_group_check=True)
                hT_new = hpool.tile([128, 128], BF16, tag=f"h{layer + 1}T")
                nc.vector.tensor_copy(hT_new[:, 0:32], htp[:, 0:32])
                nc.scalar.copy(hT_new[:, 32:128], htp[:, 32:128])
                return hT_new

            def g2_late_rounds(G2a, G2b, h2T_src, f, p3, warm):
                """G2 h2-rounds r=1..3 + CC2 + fc3-bias inject.

                warm: the f/p3 PSUM banks have completed a prior accumulation
                group, so their has_written bits are set and a DVE copy
                seeds the constants with matmuls accumulating on top
                (saves the PE inject rounds).  Cold banks (first two steps)
                use the original identity-matmul injects with start=True.
                """
                for r in range(1, 4):
                    col_round2(G2a, G2b, h2T_src[:, 32 * r:32 * (r + 1)],
                               wslice(Whh2p[r]), start=False, stop=False)
                if warm:
                    nc.vector.tensor_copy(f[:], CC2G[:])
                    nc.vector.tensor_copy(p3[:], F3rep[:])
                else:
                    col_round(f, I128[0:32, 0:32], lambda j: CC2p[:, j, :],
                              start=True, stop=False)
                    nc.tensor.matmul(p3[:], I128[0:32, 0:32], F3rep[:],
                                     start=True, stop=False,
                                     tile_position=(0, 0),
                                     skip_group_check=True)

            def g2_alloc_bias(warm):
                """alloc next G2 pair + bias inject."""
                G2a = ps_g2a.tile([128, 256], F32, tag="G2a")
                G2b = ps_g2b.tile([128, 256], F32, tag="G2b")
                if warm:
                    nc.vector.tensor_copy(G2a[:], B2G[:, 0:256])
                    nc.vector.tensor_copy(G2b[:], B2G[:, 256:512])
                else:
                    col_round2(G2a, G2b, I128[0:32, 0:32], wslice(B2rep),
                               start=True, stop=False)
                return G2a, G2b

            def g2_h2r0(G2a, G2b, h2T_src):
                """first h2 round of the next step's G2 (argmax-tail fill)."""
                col_round2(G2a, G2b, h2T_src[:, 0:32], wslice(Whh2p[0]),
                           start=False, stop=False)

            for t in range(T_STEPS):
                tb = t % 64
                if t == 0:
                    G1a = ps_g1a.tile([128, 256], F32, tag="G1a")
                    G1b = ps_g1b.tile([128, 256], F32, tag="G1b")
                    for r in range(4):
                        col_round2(G1a, G1b, h1T[:, 32 * r:32 * (r + 1)],
                                   wslice(Whh1p[r]), start=(r == 0),
                                   stop=False)
                    G2a, G2b = g2_alloc_bias(warm=False)
                    g2_h2r0(G2a, G2b, h2T)
                    f = ps_fc2.tile([128, 128], F32, tag="f")
                    p3 = ps_sm.tile([B, OUT], F32, tag="p3")

                # x path into layer 1 (K=34 onehot matmul); closes G1 with
                # the i/f half first so the sigmoid starts early
                oh_st = oh0T if t == 0 else ohT
                col_round2(G1a, G1b, oh_st[:], wslice(E1extp),
                           start=False, stop=True, close=True)
                # rest of G2/fc2/fc3 early rounds; fills PE during L1 chain
                g2_late_rounds(G2a, G2b, h2T, f, p3, warm=(t >= 2))
                # --- layer 1 chain ---
                h1T = nonlin(0, G1a, G1b, c1)
                # --- layer 2 x-part rounds (close G2) ---
                for r in range(3):
                    col_round2(G2a, G2b, h1T[:, 32 * r:32 * (r + 1)],
                               wslice(Wih2p[r]), start=False, stop=False)
                col_round2(G2a, G2b, h1T[:, 96:128], wslice(Wih2p[3]),
                           start=False, stop=True, close=True)
                # --- layer 2 chain ---
                h2T = nonlin(1, G2a, G2b, c2)
                if t == T_STEPS - 1:
                    # the loop's last Sigmoid just issued; every later ACT op
                    # (tanh/abs/copy/exp) is in the exp table, so trigger the
                    # table switch here where the 1.3us load hides under the
                    # remaining fc2/fc3 PE work instead of the postprocess
                    dummy = work.tile([B, 1], F32, tag="dummy")
                    nc.scalar.activation(dummy[:], c2[0:32, 0:1], AF.Exp)
                # --- fc2 rounds ---
                # issued BEFORE the next-step G1 h-rounds: the scheduler pops
                # ready work by program position, so fc2 (on the argmax
                # critical path) preempts G1n fill instead of queueing
                # behind it
                for r in range(4):
                    col_round(f, h2T[:, 32 * r:32 * (r + 1)],
                              lambda j, r=r: fc2Wp[:, r, j, :],
                              start=False, stop=(r == 3))
                # next step's G1 h-rounds; fill for the L2-chain/y/argmax
                # windows
                if t + 1 < T_STEPS:
                    G1na = ps_g1a.tile([128, 256], F32, tag="G1a")
                    G1nb = ps_g1b.tile([128, 256], F32, tag="G1b")
                    for r in range(4):
                        col_round2(G1na, G1nb, h1T[:, 32 * r:32 * (r + 1)],
                                   wslice(Whh1p[r]), start=(r == 0),
                                   stop=False)
                absf = work.tile([128, 128], F32, tag="absf")
                nc.scalar.activation(absf[:], f[:], AF.Abs)
                y = work.tile([128, 128], BF16, tag="y")
                nc.vector.scalar_tensor_tensor(
                    y[:], absf[:], float(0.495 / 0.505), f[:],
                    op0=ALU.mult, op1=ALU.add)
                ytp = ps_ht.tile([128, 128], BF16, tag="htp")
                nc.tensor.matmul(ytp[:], y[:], I128[:], is_transpose=True,
                                 skip_group_check=True)
                yT = work.tile([128, 128], BF16, tag="yT")
                nc.scalar.copy(yT[:], ytp[:])
                # --- fc3 rounds -> [32, 33] ---
                p3_cur, f_cur = p3, f
                for r in range(4):
                    nc.tensor.matmul(p3_cur[:], yT[:, 32 * r:32 * (r + 1)],
                                     fc3Wp[:, r, :], start=False, stop=(r == 3),
                                     tile_position=(0, 0), skip_group_check=True)
                # next step's G2 bias + first h2-round; fills the argmax tail
                if t + 1 < T_STEPS:
                    G2a, G2b = g2_alloc_bias(warm=(t + 1 >= 2))
                    g2_h2r0(G2a, G2b, h2T)
                    G1a, G1b = G1na, G1nb
                    f = ps_fc2.tile([128, 128], F32, tag="f")
                    p3 = ps_sm.tile([B, OUT], F32, tag="p3")
                # --- argmax feedback ---
                if t < T_STEPS - 1:
                    mx = work.tile([B, 8], F32, tag="mx")
                    nc.vector.max(mx[:], p3_cur[:, 0:32])
                    oh = work.tile([B, 32], BF16, tag="oh")
                    nc.vector.tensor_scalar(oh[:], p3_cur[:, 0:32],
                                            mx[:, 0:1], None, op0=ALU.is_equal)
                    nc.vector.transpose(ohT[0:32, :], oh[:])
                nc.scalar.copy(predbuf[:, tb, :], p3_cur[:])

            # gate tile: written after the loop's last step; used as a zero
            # bias on the chunked exps below so the scheduler cannot hoist
            # them into the loop (which would thrash the ACT tables)
            gate0 = work.tile([B, 1], F32, tag="gate0")
            nc.vector.tensor_scalar(gate0[:], predbuf[:, T_STEPS - 1, 0:1],
                                    0.0, None, op0=ALU.mult)

            # ---- postprocess (exp/reduce in gated chunks; ACT/DVE pipeline) ----
            e = state.tile([B, 64, OUT], F32, tag="e")
            s = work.tile([B, 64], F32, tag="s")
            for t0 in range(0, 64, 32):
                nc.scalar.activation(e[:, t0:t0 + 32, :],
                                     predbuf[:, t0:t0 + 32, :], AF.Exp,
                                     bias=gate0[:, 0:1])
                nc.vector.tensor_reduce(s[:, t0:t0 + 32],
                                        e[:, t0:t0 + 32, 0:32],
                                        mybir.AxisListType.X, ALU.add)
            lns = work.tile([B, 64], F32, tag="lns")
            nc.scalar.activation(lns[:, 0:32], s[:, 0:32], AF.Ln)
            nc.scalar.activation(lns[:, 32:64], s[:, 32:64], AF.Ln)
            outf = state.tile([B, 64, OUT], F32, tag="outf")
            # duration softmax over time; final multiply on gpsimd so it
            # overlaps the DVE subtracts below
            sd = work.tile([B, 1], F32, tag="sd")
            nc.vector.tensor_reduce(sd[:], e[:, :, 32:33], mybir.AxisListType.XY,
                                    ALU.add)
            rsd = work.tile([B, 1], F32, tag="rsd")
            nc.vector.reciprocal(rsd[:], sd[:])
            nc.gpsimd.tensor_scalar(outf[:, :, 32:33], e[:, :, 32:33],
                                    rsd[:, 0:1], None, op0=ALU.mult)
            # log-probs in time-chunks alternating DVE/gpsimd; each chunk
            # DMAs out contiguously (all 33 channels) while others compute
            for i, t0 in enumerate(range(0, 64, 16)):
                eng = nc.vector if i % 2 == 0 else nc.gpsimd
                eng.tensor_tensor(
                    outf[:, t0:t0 + 16, 0:32], predbuf[:, t0:t0 + 16, 0:32],
                    lns[:, t0:t0 + 16].broadcast_to((B, 16, 32)),
                    ALU.subtract)
                (nc.sync if i % 2 == 0 else nc.scalar).dma_start(
                    out_d[:, t0:t0 + 16, :], outf[:, t0:t0 + 16, :])

    nc.compile()
    return nc, out_d.tensor.name


def kernel(**inputs):
    from concourse import bass_utils

    g, per_core = _prep(inputs)
    if "prog" not in _PROGRAM_CACHE:
        _PROGRAM_CACHE["prog"] = _build_program()
    nc, out_name = _PROGRAM_CACHE["prog"]

    bf16 = _bf16np()
    in_maps = []
    for ci in range(N_CORES):
        m = dict(g)
        m.update(per_core[ci])
        in_maps.append({k: np.ascontiguousarray(
            np.asarray(v, np.float32).astype(bf16)
            if k in _BF16_NAMES else np.asarray(v, np.float32))
            for k, v in m.items()})
    ncores = int(os.environ.get("KERNEL_CORES", str(N_CORES)))
    kwargs = {}
    if os.environ.get("KERNEL_TRACE"):
        kwargs = dict(trace=True, tmpdir=os.environ.get("KERNEL_TRACE_DIR") or None)
    res = bass_utils.run_bass_kernel_spmd(nc, in_maps[:ncores],
                                          core_ids=list(range(ncores)), **kwargs)
    global LAST_EXEC_NS
    LAST_EXEC_NS = res.exec_time_ns
    out = np.concatenate([r[out_name] for r in res.results], axis=0)
    return out.astype(np.float32)


# revision 61
# speedup vs baseline: 1.0017x; 1.0017x over previous
"""Trainium2 Bass kernel for nn_Decoder_76974403879078.

2-layer LSTM decoder, B=256, H=512, T=64 steps, argmax feedback.
Sharding: data-parallel over batch, 8 cores x 32.

Device design (per core, batch M=32):
  - All matmul operands bf16 (1 cycle/row on the PE vs 4 for fp32);
    accumulation stays fp32 in PSUM.  Elementwise/LSTM state fp32.
    Host-side bf16 study: rel err 3.7e-4 vs fp32 reference (tolerance 2e-2).
  - Moving operand = weights, stationary = h^T k-tiles [128,32]; 4-way PE
    column tiling writes the four gate chunks to PSUM partition groups
    32j..32j+32.
  - Gate layout "interleaved": chunk j (partitions 32j:32j+32) holds free
    cols [i|f|o|g] for hidden slice 128j:128(j+1), 128 each.
  - Each gate PSUM is a PAIR of tiles: Ga = i,f cols (0:256), Gb = o,g
    (256:512).  Dependency tracking is PSUM-tile-granular, so splitting
    lets the i/f sigmoid fire as soon as the a-half closes (~400ns before
    the full gate matmul finishes); the closing round orders a-halves
    first.  All gate rounds issue as [128,256] half-MMs (same streamed
    rows, so same PE time).
  - x-path folded: x_t = [emb[a], dur] enters layer 1 via a K=34 matmul with
    stationary onehot^T (+dur row +bias row) against a host-precomputed
    E1ext = [emb@Wih1[:,:511]^T ; Wih1[:,511] ; bih1+bhh1] table.
  - Constant injects (layer-2 gate bias B2G, conditionals path CC2G, fc3
    bias F3rep) are DVE copies into "warm" PSUM banks: after a bank's first
    accumulation group its has_written bits stay set, so start=False
    matmuls accumulate onto engine-written seeds (verified on HW).  The
    first two steps (cold banks) use identity-matmul injects instead.
    Saves ~1.1us/step of PE inject rounds.
  - Program order tuned for the greedy ready-first scheduler: fc2 is
    issued before the next-step G1 h-rounds so the argmax critical path
    preempts fill work; the Whh2-h2 rounds are the fill reserved for the
    L1-chain window.
  - argmax feedback: DVE max -> tensor_scalar is_equal -> DVE 32x32 transpose
    gives onehot^T for the next step's K=34 matmul.
  - log_softmax / time-softmax postprocess: chunked exp/reduce gated on the
    last step (a zero bias AP blocks scheduler hoisting, which would thrash
    ACT function tables); the exp-table switch is triggered right after the
    loop's last sigmoid so the 1.3us table load hides under PE work;
    output DMAd in contiguous 16-step chunks overlapping the subtracts.
  - weights stored in DRAM as bf16 (6.3MB/core); per-k-tile weight tiles
    DMAd in first-use order across three engine queues so the time loop
    starts after ~1MB instead of the full load.

Measured (CoreSim TRN2 cost model, per core): 820.4us vs 3439.6us for the
fp32 predecessor (4.2x).  PE busy 95%.  Backend-validated rel err 3.74e-4.
"""
import sys
import numpy as np

sys.path.insert(0, "/opt/trn_rl_repo")

import os
HIDDEN = 512
OUT = 33
T_STEPS = int(os.environ.get("KERNEL_STEPS", "64"))
B_FULL = 256
N_CORES = 8
B = B_FULL // N_CORES  # 32
SLOPE = 0.01

_PROGRAM_CACHE = {}
LAST_EXEC_NS = None

# tensors stored/streamed as bf16 on device
_BF16_NAMES = {"Whh1p", "Wih2p", "Whh2p", "E1extp", "B2rep", "fc2Wp",
               "fc3Wp", "F3rep", "I128", "oh0T", "h1T0", "h2T0", "CC2p",
               "B2G", "CC2G"}


def _bf16np():
    import ml_dtypes
    return ml_dtypes.bfloat16


def _gate_perm():
    """perm[j*512 + q*128 + n] = original gate-row index.

    chunk j free layout: [i(0:128) | f(128:256) | o(256:384) | g(384:512)],
    hidden slice j = 128j..128(j+1).  torch gate order in W: i,f,g,o.
    """
    base = {"i": 0, "f": 512, "g": 1024, "o": 1536}
    perm = []
    for j in range(4):
        for q in ("i", "f", "o", "g"):
            perm.extend(range(base[q] + 128 * j, base[q] + 128 * (j + 1)))
    return np.asarray(perm, np.int64)


def _pack_w(Wl, perm):
    """[2048,512] gate weight -> rhs tile [128, 4(k-tile), 2048(perm cols)]."""
    wt = Wl[perm].T.astype(np.float32)            # [512 hidden, 2048 perm]
    return np.ascontiguousarray(
        wt.reshape(4, 128, 2048).transpose(1, 0, 2))  # [128,4,2048]


def _prep(inputs):
    """Host-side packing. Returns dict of global consts + per-core arrays."""
    f32 = np.float32
    emb = np.asarray(inputs["emb"], f32)
    Wih = np.asarray(inputs["Wih"], f32)
    Whh = np.asarray(inputs["Whh"], f32)
    bih = np.asarray(inputs["bih"], f32)
    bhh = np.asarray(inputs["bhh"], f32)
    fcW = np.asarray(inputs["fcW"], f32)
    fcb = np.asarray(inputs["fcb"], f32)
    fc2W = np.asarray(inputs["fc2W"], f32)
    fc2b = np.asarray(inputs["fc2b"], f32)
    fc3W = np.asarray(inputs["fc3W"], f32)
    fc3b = np.asarray(inputs["fc3b"], f32)
    h0 = np.asarray(inputs["h0"], f32)
    c0 = np.asarray(inputs["c0"], f32)
    conditionals = np.asarray(inputs["conditionals"], f32)

    perm = _gate_perm()
    g = {}
    g["Whh1p"] = _pack_w(Whh[0], perm)
    g["Wih2p"] = _pack_w(Wih[1], perm)
    g["Whh2p"] = _pack_w(Whh[1], perm)

    Wih1perm = Wih[0][perm]                       # [2048, 512]
    e1 = np.zeros((34, 2048), f32)
    e1[:32] = emb @ Wih1perm[:, :511].T
    e1[32] = Wih1perm[:, 511]
    e1[33] = (bih[0] + bhh[0])[perm]
    g["E1extp"] = e1

    b2p = (bih[1] + bhh[1])[perm].astype(f32)
    g["B2rep"] = np.tile(b2p[None, :], (B, 1)).astype(f32)
    # bias pre-laid-out in G2 PSUM shape: row 32j+b = bias[512j:512j+512]
    g["B2G"] = np.ascontiguousarray(
        np.repeat(b2p.reshape(4, 1, 512), B, axis=1).reshape(128, 512))

    g["fc2Wp"] = np.ascontiguousarray(
        fc2W.T.reshape(4, 128, 4, 128).transpose(1, 0, 2, 3))   # [128,4,4,128]
    # leaky(x) computed on device as 0.505x + 0.495|x| = 0.505*(x + k|x|);
    # the 0.505 is folded here into fc3W
    g["fc3Wp"] = np.ascontiguousarray(
        fc3W.T.reshape(4, 128, OUT).transpose(1, 0, 2)) * np.float32(0.505)
    g["F3rep"] = np.tile(fc3b[None, :], (B, 1)).astype(f32)

    g["I128"] = np.eye(128, dtype=f32)

    oh0 = np.zeros((34, B), f32)
    oh0[0, :] = 1.0   # SOS = 0
    oh0[32, :] = 0.0  # dur at t=0
    oh0[33, :] = 1.0  # bias row
    g["oh0T"] = oh0

    cond = conditionals @ fcW.T + fcb
    cond = np.where(cond >= 0, cond, SLOPE * cond).astype(f32)
    CC2 = (cond @ fc2W.T + fc2b).astype(f32)      # [256, 512]

    per_core = []
    for ci in range(N_CORES):
        sl = slice(ci * B, (ci + 1) * B)
        pc = {}
        for l, name in ((0, "h1T0"), (1, "h2T0")):
            hc = h0[l, sl]                         # [32, 512]
            pc[name] = np.ascontiguousarray(
                hc.reshape(B, 4, 128).transpose(2, 1, 0).reshape(128, 128))
        for l, name in ((0, "c10"), (1, "c20")):
            cc = c0[l, sl]
            pc[name] = np.ascontiguousarray(
                cc.reshape(B, 4, 128).transpose(1, 0, 2).reshape(128, 128))
        pc["CC2p"] = np.ascontiguousarray(CC2[sl].reshape(B, 4, 128))
        # CC2 pre-laid-out in fc2 PSUM shape [32j+b, m]
        pc["CC2G"] = np.ascontiguousarray(
            CC2[sl].reshape(B, 4, 128).transpose(1, 0, 2).reshape(128, 128))
        per_core.append(pc)
    return g, per_core


# ---------------------------------------------------------------------------
# numpy emulation of the exact device algorithm (for layout validation)
# ---------------------------------------------------------------------------
def _emulate_core(g, pc):
    f32 = np.float32

    def sig(x):
        return (1.0 / (1.0 + np.exp(-x))).astype(f32)

    h1T, h2T = pc["h1T0"].copy(), pc["h2T0"].copy()
    c1, c2 = pc["c10"].copy(), pc["c20"].copy()
    ohT = g["oh0T"].copy()
    preds = np.zeros((B, T_STEPS, OUT), f32)

    for t in range(T_STEPS):
        for layer in range(2):
            G = np.zeros((128, 512), f32)
            if layer == 0:
                Wp = g["Whh1p"]
                hT = h1T
                for j in range(4):
                    G[32 * j:32 * (j + 1)] += ohT.T @ g["E1extp"][:, 512 * j:512 * (j + 1)]
            else:
                Wp = g["Whh2p"]
                hT = h2T
                for j in range(4):
                    G[32 * j:32 * (j + 1)] += g["B2rep"][:, 512 * j:512 * (j + 1)]
                    for r in range(4):
                        G[32 * j:32 * (j + 1)] += (
                            h1T[:, 32 * r:32 * (r + 1)].T @ g["Wih2p"][:, r, 512 * j:512 * (j + 1)])
            for j in range(4):
                for r in range(4):
                    G[32 * j:32 * (j + 1)] += (
                        hT[:, 32 * r:32 * (r + 1)].T @ Wp[:, r, 512 * j:512 * (j + 1)])
            sg = np.empty_like(G)
            sg[:, 0:384] = sig(G[:, 0:384])
            sg[:, 384:512] = np.tanh(G[:, 384:512])
            c = c1 if layer == 0 else c2
            cn = sg[:, 128:256] * c + sg[:, 0:128] * sg[:, 384:512]
            hp = sg[:, 256:384] * np.tanh(cn)
            hT_new = np.zeros((128, 128), f32)
            for j in range(4):
                hT_new[:, 32 * j:32 * (j + 1)] = hp[32 * j:32 * (j + 1), :].T
            if layer == 0:
                c1, h1T = cn, hT_new
            else:
                c2, h2T = cn, hT_new
        # fc2 (packed out [128,128]) + CC2
        f = np.zeros((128, 128), f32)
        for j in range(4):
            f[32 * j:32 * (j + 1)] = pc["CC2p"][:, j, :]
            for r in range(4):
                f[32 * j:32 * (j + 1)] += (
                    h2T[:, 32 * r:32 * (r + 1)].T @ g["fc2Wp"][:, r, j, :])
        K_ABS = f32(0.495 / 0.505)
        y = (f + K_ABS * np.abs(f)).astype(f32)
        yT = np.zeros((128, 128), f32)
        for j in range(4):
            yT[:, 32 * j:32 * (j + 1)] = y[32 * j:32 * (j + 1), :].T
        pred = g["F3rep"].copy()
        for r in range(4):
            pred += yT[:, 32 * r:32 * (r + 1)].T @ g["fc3Wp"][:, r, :]
        preds[:, t, :] = pred
        if t < T_STEPS - 1:
            mx = pred[:, :32].max(1, keepdims=True)
            oh = (pred[:, :32] == mx).astype(f32)
            ohT[0:32, :] = oh.T
            ohT[32, :] = 1.0
            ohT[33, :] = 1.0
    # postprocess
    e = np.exp(preds)
    s = e[:, :, :32].sum(-1)
    logp = preds[:, :, :32] - np.log(s)[:, :, None]
    sd = e[:, :, 32].sum(1, keepdims=True)
    dur = e[:, :, 32] / sd
    return np.concatenate([logp, dur[:, :, None]], axis=-1).astype(np.float32)


def emulate(inputs):
    g, per_core = _prep(inputs)
    outs = [_emulate_core(g, pc) for pc in per_core]
    return np.concatenate(outs, axis=0)


# ---------------------------------------------------------------------------
# Bass program
# ---------------------------------------------------------------------------
def _build_program():
    import concourse.bass as bass
    import concourse.tile as tile
    from concourse import mybir, bacc

    F32 = mybir.dt.float32
    BF16 = mybir.dt.bfloat16
    AF = mybir.ActivationFunctionType
    ALU = mybir.AluOpType

    nc = bacc.Bacc("TRN2", target_bir_lowering=False, debug=False)

    def din(name, shape):
        dt = BF16 if name in _BF16_NAMES else F32
        return nc.dram_tensor(name, list(shape), dt, kind="ExternalInput").ap()

    d = {
        "Whh1p": din("Whh1p", (128, 4, 2048)),
        "Wih2p": din("Wih2p", (128, 4, 2048)),
        "Whh2p": din("Whh2p", (128, 4, 2048)),
        "E1extp": din("E1extp", (34, 2048)),
        "fc2Wp": din("fc2Wp", (128, 4, 4, 128)),
        "fc3Wp": din("fc3Wp", (128, 4, OUT)),
        "F3rep": din("F3rep", (B, OUT)),
        "I128": din("I128", (128, 128)),
        "oh0T": din("oh0T", (34, B)),
        "h1T0": din("h1T0", (128, 128)),
        "h2T0": din("h2T0", (128, 128)),
        "c10": din("c10", (128, 128)),
        "c20": din("c20", (128, 128)),
        "B2G": din("B2G", (128, 512)),
        "CC2G": din("CC2G", (128, 128)),
    }
    out_d = nc.dram_tensor("out", [B, 64, OUT], F32, kind="ExternalOutput").ap()

    with tile.TileContext(nc) as tc:
        import contextlib
        ctx = contextlib.ExitStack()
        with ctx:
            consts = ctx.enter_context(tc.tile_pool(name="consts", bufs=1))
            state = ctx.enter_context(tc.tile_pool(name="state", bufs=1))
            work = ctx.enter_context(tc.tile_pool(name="work", bufs=2))
            hpool = ctx.enter_context(tc.tile_pool(name="hpool", bufs=2))
            ps_g1a = ctx.enter_context(tc.tile_pool(name="ps_g1a", bufs=1, space="PSUM"))
            ps_g1b = ctx.enter_context(tc.tile_pool(name="ps_g1b", bufs=1, space="PSUM"))
            ps_g2a = ctx.enter_context(tc.tile_pool(name="ps_g2a", bufs=1, space="PSUM"))
            ps_g2b = ctx.enter_context(tc.tile_pool(name="ps_g2b", bufs=1, space="PSUM"))
            ps_fc2 = ctx.enter_context(tc.tile_pool(name="ps_fc2", bufs=2, space="PSUM"))
            ps_sm = ctx.enter_context(tc.tile_pool(name="ps_sm", bufs=1, space="PSUM"))
            ps_ht = ctx.enter_context(tc.tile_pool(name="ps_ht", bufs=1, space="PSUM"))

            # ---- constant tiles (bf16 streams, fp32 state) ----
            # gate weights split per k-tile so each DMA chunk unblocks its
            # own rounds (tile-granular dependencies)
            Whh1p = [consts.tile([128, 2048], BF16, name=f"Whh1p{r}")
                     for r in range(4)]
            Wih2p = [consts.tile([128, 2048], BF16, name=f"Wih2p{r}")
                     for r in range(4)]
            Whh2p = [consts.tile([128, 2048], BF16, name=f"Whh2p{r}")
                     for r in range(4)]
            E1extp = consts.tile([34, 2048], BF16)
            B2G = consts.tile([128, 512], BF16)
            CC2G = consts.tile([128, 128], BF16)
            fc2Wp = consts.tile([128, 4, 4, 128], BF16)
            fc3Wp = consts.tile([128, 4, OUT], BF16)
            F3rep = consts.tile([B, OUT], BF16)
            I128 = consts.tile([128, 128], BF16)
            oh0T = consts.tile([34, B], BF16)

            c1 = state.tile([128, 128], F32, tag="c1")
            c2 = state.tile([128, 128], F32, tag="c2")
            h1T = hpool.tile([128, 128], BF16, tag="h1T")
            h2T = hpool.tile([128, 128], BF16, tag="h2T")

            # DMAs ordered by first use AND spread across three engine
            # queues so the three weight streams load concurrently.
            nc.sync.dma_start(Whh1p[0][:], d["Whh1p"][:, 0])
            nc.sync.dma_start(h1T[:], d["h1T0"])
            nc.sync.dma_start(oh0T[:], d["oh0T"])
            nc.sync.dma_start(E1extp[:], d["E1extp"])
            nc.sync.dma_start(c1[:], d["c10"])
            for r in range(1, 4):
                nc.sync.dma_start(Whh1p[r][:], d["Whh1p"][:, r])
            nc.gpsimd.dma_start(h2T[:], d["h2T0"])
            nc.gpsimd.dma_start(c2[:], d["c20"])
            for r in range(4):
                nc.gpsimd.dma_start(Whh2p[r][:], d["Whh2p"][:, r])
            nc.scalar.dma_start(I128[:], d["I128"])
            nc.scalar.dma_start(F3rep[:], d["F3rep"])
            nc.scalar.dma_start(B2G[:], d["B2G"])
            nc.scalar.dma_start(CC2G[:], d["CC2G"])
            for r in range(4):
                nc.scalar.dma_start(Wih2p[r][:], d["Wih2p"][:, r])
            nc.scalar.dma_start(fc2Wp[:], d["fc2Wp"])
            nc.scalar.dma_start(fc3Wp[:], d["fc3Wp"])

            ohT = state.tile([34, B], BF16, tag="ohT")
            nc.vector.memset(ohT[32:34, :], 1.0)

            predbuf = state.tile([B, 64, OUT], F32, tag="predbuf")
            if T_STEPS < 64:
                nc.vector.memset(predbuf[:], 0.0)

            def col_round(psum, lhsT, rhs_fn, start, stop):
                for j in range(4):
                    nc.tensor.matmul(
                        psum[32 * j:32 * (j + 1), :], lhsT, rhs_fn(j),
                        start=start, stop=stop, tile_position=(0, 32 * j),
                        skip_group_check=True,
                    )

            def col_round2(Ga, Gb, lhsT, rhs_fn, start, stop, close=False):
                """One k-round split into a (gate cols 0:256 = i,f) and b
                (256:512 = o,g) half-MMs targeting separate PSUM tiles, so
                the sigmoid's dependency cone closes with the a-half.
                close=True orders all a-halves first."""
                order = ([(h, j) for h in (0, 1) for j in range(4)] if close
                         else [(h, j) for j in range(4) for h in (0, 1)])
                for h, j in order:
                    nc.tensor.matmul(
                        (Ga if h == 0 else Gb)[32 * j:32 * (j + 1), :], lhsT,
                        rhs_fn(j, h), start=start, stop=stop,
                        tile_position=(0, 32 * j), skip_group_check=True)

            def wslice(W):
                return lambda j, h: W[:, 512 * j + 256 * h:512 * j + 256 * (h + 1)]

            def nonlin(layer, Ga, Gb, c_own):
                """sigmoid/tanh + c/h update + transpose; returns new h^T.

                The PSUM->SBUF evacuation of h^T is split into column
                slices so the first dependent matmul round starts early.
                """
                sg = work.tile([128, 512], F32, tag=f"sg{layer}")
                # i,f sigmoid first, then tanh(g) (unblocks t1/t2 sooner);
                # the o-gate sigmoid is only needed later for hp
                nc.scalar.activation(sg[:, 0:256], Ga[:], AF.Sigmoid)
                nc.scalar.activation(sg[:, 384:512], Gb[:, 128:256], AF.Tanh)
                nc.scalar.activation(sg[:, 256:384], Gb[:, 0:128], AF.Sigmoid)
                t1 = work.tile([128, 128], F32, tag="t1")
                t2 = work.tile([128, 128], F32, tag="t2")
                nc.vector.tensor_tensor(t1[:], sg[:, 0:128], sg[:, 384:512], ALU.mult)
                nc.vector.tensor_tensor(t2[:], sg[:, 128:256], c_own[:], ALU.mult)
                nc.vector.tensor_tensor(c_own[:], t1[:], t2[:], ALU.add)
                tc_t = work.tile([128, 128], F32, tag="tc")
                nc.scalar.activation(tc_t[:], c_own[:], AF.Tanh)
                hp = work.tile([128, 128], BF16, tag=f"hp{layer}")
                nc.vector.tensor_tensor(hp[:], sg[:, 256:384], tc_t[:], ALU.mult)
                htp = ps_ht.tile([128, 128], BF16, tag="htp")
                nc.tensor.matmul(htp[:], hp[:], I128[:], is_transpose=True,
                                 skip_group_check=True)
                hT_new = hpool.tile([128, 128], BF16, tag=f"h{layer + 1}T")
                nc.vector.tensor_copy(hT_new[:, 0:32], htp[:, 0:32])
                nc.scalar.copy(hT_new[:, 32:128], htp[:, 32:128])
                return hT_new

            def g2_late_rounds(G2a, G2b, h2T_src, f, p3, warm):
                """G2 h2-rounds r=1..3 + CC2 + fc3-bias inject.

                warm: the f/p3 PSUM banks have completed a prior accumulation
                group, so their has_written bits are set and a DVE copy
                seeds the constants with matmuls accumulating on top
                (saves the PE inject rounds).  Cold banks (first two steps)
                use the original identity-matmul injects with start=True.
                """
                for r in range(1, 4):
                    col_round2(G2a, G2b, h2T_src[:, 32 * r:32 * (r + 1)],
                               wslice(Whh2p[r]), start=False, stop=False)
                nc.vector.tensor_copy(f[:], CC2G[:])
                nc.vector.tensor_copy(p3[:], F3rep[:])

            def g2_alloc_bias(warm):
                """alloc next G2 pair + bias inject."""
                G2a = ps_g2a.tile([128, 256], F32, tag="G2a")
                G2b = ps_g2b.tile([128, 256], F32, tag="G2b")
                nc.vector.tensor_copy(G2a[:], B2G[:, 0:256])
                nc.vector.tensor_copy(G2b[:], B2G[:, 256:512])
                return G2a, G2b

            def g2_h2r0(G2a, G2b, h2T_src):
                """first h2 round of the next step's G2 (argmax-tail fill)."""
                col_round2(G2a, G2b, h2T_src[:, 0:32], wslice(Whh2p[0]),
                           start=False, stop=False)

            # Pre-warm the warm-seeded PSUM banks during the startup DMA
            # wait: a dummy full-region accumulation group sets each bank's
            # has_written bits, so every loop step can use the cheap
            # DVE-copy inject path (no cold identity-matmul rounds).
            def warm_bank(tile_ap, n):
                for j in range(4):
                    nc.tensor.matmul(tile_ap[32 * j:32 * (j + 1), :],
                                     I128[0:32, 0:32], E1extp[0:32, 0:n],
                                     start=True, stop=True,
                                     tile_position=(0, 32 * j),
                                     skip_group_check=True)

            wg2a = ps_g2a.tile([128, 256], F32, tag="G2a")
            wg2b = ps_g2b.tile([128, 256], F32, tag="G2b")
            warm_bank(wg2a, 256)
            warm_bank(wg2b, 256)
            for _ in range(2):
                wf = ps_fc2.tile([128, 128], F32, tag="f")
                warm_bank(wf, 128)
            wp3 = ps_sm.tile([B, OUT], F32, tag="p3")
            nc.tensor.matmul(wp3[:], I128[0:32, 0:32], E1extp[0:32, 0:OUT],
                             start=True, stop=True, tile_position=(0, 0),
                             skip_group_check=True)

            for t in range(T_STEPS):
                tb = t % 64
                if t == 0:
                    G1a = ps_g1a.tile([128, 256], F32, tag="G1a")
                    G1b = ps_g1b.tile([128, 256], F32, tag="G1b")
                    for r in range(4):
                        col_round2(G1a, G1b, h1T[:, 32 * r:32 * (r + 1)],
                                   wslice(Whh1p[r]), start=(r == 0),
                                   stop=False)
                    G2a, G2b = g2_alloc_bias(warm=True)
                    g2_h2r0(G2a, G2b, h2T)
                    f = ps_fc2.tile([128, 128], F32, tag="f")
                    p3 = ps_sm.tile([B, OUT], F32, tag="p3")

                # x path into layer 1 (K=34 onehot matmul); closes G1 with
                # the i/f half first so the sigmoid starts early
                oh_st = oh0T if t == 0 else ohT
                col_round2(G1a, G1b, oh_st[:], wslice(E1extp),
                           start=False, stop=True, close=True)
                # rest of G2/fc2/fc3 early rounds; fills PE during L1 chain
                g2_late_rounds(G2a, G2b, h2T, f, p3, warm=True)
                # --- layer 1 chain ---
                h1T = nonlin(0, G1a, G1b, c1)
                # --- layer 2 x-part rounds (close G2) ---
                for r in range(3):
                    col_round2(G2a, G2b, h1T[:, 32 * r:32 * (r + 1)],
                               wslice(Wih2p[r]), start=False, stop=False)
                col_round2(G2a, G2b, h1T[:, 96:128], wslice(Wih2p[3]),
                           start=False, stop=True, close=True)
                # --- layer 2 chain ---
                h2T = nonlin(1, G2a, G2b, c2)
                if t == T_STEPS - 1:
                    # the loop's last Sigmoid just issued; every later ACT op
                    # (tanh/abs/copy/exp) is in the exp table, so trigger the
                    # table switch here where the 1.3us load hides under the
                    # remaining fc2/fc3 PE work instead of the postprocess
                    dummy = work.tile([B, 1], F32, tag="dummy")
                    nc.scalar.activation(dummy[:], c2[0:32, 0:1], AF.Exp)
                # --- fc2 rounds ---
                # issued BEFORE the next-step G1 h-rounds: the scheduler pops
                # ready work by program position, so fc2 (on the argmax
                # critical path) preempts G1n fill instead of queueing
                # behind it
                for r in range(4):
                    col_round(f, h2T[:, 32 * r:32 * (r + 1)],
                              lambda j, r=r: fc2Wp[:, r, j, :],
                              start=False, stop=(r == 3))
                # next step's G1 h-rounds; fill for the L2-chain/y/argmax
                # windows
                if t + 1 < T_STEPS:
                    G1na = ps_g1a.tile([128, 256], F32, tag="G1a")
                    G1nb = ps_g1b.tile([128, 256], F32, tag="G1b")
                    for r in range(4):
                        col_round2(G1na, G1nb, h1T[:, 32 * r:32 * (r + 1)],
                                   wslice(Whh1p[r]), start=(r == 0),
                                   stop=False)
                absf = work.tile([128, 128], F32, tag="absf")
                nc.scalar.activation(absf[:], f[:], AF.Abs)
                y = work.tile([128, 128], BF16, tag="y")
                nc.vector.scalar_tensor_tensor(
                    y[:], absf[:], float(0.495 / 0.505), f[:],
                    op0=ALU.mult, op1=ALU.add)
                ytp = ps_ht.tile([128, 128], BF16, tag="htp")
                nc.tensor.matmul(ytp[:], y[:], I128[:], is_transpose=True,
                                 skip_group_check=True)
                yT = work.tile([128, 128], BF16, tag="yT")
                nc.scalar.copy(yT[:], ytp[:])
                # --- fc3 rounds -> [32, 33] ---
                p3_cur, f_cur = p3, f
                for r in range(4):
                    nc.tensor.matmul(p3_cur[:], yT[:, 32 * r:32 * (r + 1)],
                                     fc3Wp[:, r, :], start=False, stop=(r == 3),
                                     tile_position=(0, 0), skip_group_check=True)
                # next step's G2 bias + first h2-round; fills the argmax tail
                if t + 1 < T_STEPS:
                    G2a, G2b = g2_alloc_bias(warm=True)
                    g2_h2r0(G2a, G2b, h2T)
                    G1a, G1b = G1na, G1nb
                    f = ps_fc2.tile([128, 128], F32, tag="f")
                    p3 = ps_sm.tile([B, OUT], F32, tag="p3")
                # --- argmax feedback ---
                if t < T_STEPS - 1:
                    mx = work.tile([B, 8], F32, tag="mx")
                    nc.vector.max(mx[:], p3_cur[:, 0:32])
                    oh = work.tile([B, 32], BF16, tag="oh")
                    nc.vector.tensor_scalar(oh[:], p3_cur[:, 0:32],
                                            mx[:, 0:1], None, op0=ALU.is_equal)
                    nc.vector.transpose(ohT[0:32, :], oh[:])
                nc.scalar.copy(predbuf[:, tb, :], p3_cur[:])

            # gate tile: written after the loop's last step; used as a zero
            # bias on the chunked exps below so the scheduler cannot hoist
            # them into the loop (which would thrash the ACT tables)
            gate0 = work.tile([B, 1], F32, tag="gate0")
            nc.vector.tensor_scalar(gate0[:], predbuf[:, T_STEPS - 1, 0:1],
                                    0.0, None, op0=ALU.mult)

            # ---- postprocess (exp/reduce in gated chunks; ACT/DVE pipeline) ----
            e = state.tile([B, 64, OUT], F32, tag="e")
            s = work.tile([B, 64], F32, tag="s")
            for t0 in range(0, 64, 32):
                nc.scalar.activation(e[:, t0:t0 + 32, :],
                                     predbuf[:, t0:t0 + 32, :], AF.Exp,
                                     bias=gate0[:, 0:1])
                nc.vector.tensor_reduce(s[:, t0:t0 + 32],
                                        e[:, t0:t0 + 32, 0:32],
                                        mybir.AxisListType.X, ALU.add)
            lns = work.tile([B, 64], F32, tag="lns")
            nc.scalar.activation(lns[:, 0:32], s[:, 0:32], AF.Ln)
            nc.scalar.activation(lns[:, 32:64], s[:, 32:64], AF.Ln)
            outf = state.tile([B, 64, OUT], F32, tag="outf")
            # duration softmax over time; final multiply on gpsimd so it
            # overlaps the DVE subtracts below
            sd = work.tile([B, 1], F32, tag="sd")
            nc.vector.tensor_reduce(sd[:], e[:, :, 32:33], mybir.AxisListType.XY,
                                    ALU.add)
            rsd = work.tile([B, 1], F32, tag="rsd")
            nc.vector.reciprocal(rsd[:], sd[:])
            nc.gpsimd.tensor_scalar(outf[:, :, 32:33], e[:, :, 32:33],
                                    rsd[:, 0:1], None, op0=ALU.mult)
            # log-probs in time-chunks alternating DVE/gpsimd; each chunk
            # DMAs out contiguously (all 33 channels) while others compute
            for i, t0 in enumerate(range(0, 64, 16)):
                eng = nc.vector if i % 2 == 0 else nc.gpsimd
                eng.tensor_tensor(
                    outf[:, t0:t0 + 16, 0:32], predbuf[:, t0:t0 + 16, 0:32],
                    lns[:, t0:t0 + 16].broadcast_to((B, 16, 32)),
                    ALU.subtract)
                (nc.sync if i % 2 == 0 else nc.scalar).dma_start(
                    out_d[:, t0:t0 + 16, :], outf[:, t0:t0 + 16, :])

    nc.compile()
    return nc, out_d.tensor.name


def kernel(**inputs):
    from concourse import bass_utils

    g, per_core = _prep(inputs)
    if "prog" not in _PROGRAM_CACHE:
        _PROGRAM_CACHE["prog"] = _build_program()
    nc, out_name = _PROGRAM_CACHE["prog"]

    bf16 = _bf16np()
    in_maps = []
    for ci in range(N_CORES):
        m = dict(g)
        m.update(per_core[ci])
        in_maps.append({k: np.ascontiguousarray(
            np.asarray(v, np.float32).astype(bf16)
            if k in _BF16_NAMES else np.asarray(v, np.float32))
            for k, v in m.items()})
    ncores = int(os.environ.get("KERNEL_CORES", str(N_CORES)))
    kwargs = {}
    if os.environ.get("KERNEL_TRACE"):
        kwargs = dict(trace=True, tmpdir=os.environ.get("KERNEL_TRACE_DIR") or None)
    res = bass_utils.run_bass_kernel_spmd(nc, in_maps[:ncores],
                                          core_ids=list(range(ncores)), **kwargs)
    global LAST_EXEC_NS
    LAST_EXEC_NS = res.exec_time_ns
    out = np.concatenate([r[out_name] for r in res.results], axis=0)
    return out.astype(np.float32)
